# revision 28
# baseline (speedup 1.0000x reference)
"""DAGNN (10-hop propagation + sigmoid gating) Bass kernel for 8 trn2 NeuronCores.

Strategy (1D node partition, SPMD-uniform schedule), v2:
  - Host assigns nodes to (core, window, slot) with degree balancing so every
    core runs an identical instruction stream (one NEFF, 8 cores).
  - Replica layout is WINDOW-CHUNK major: the 104 windows per core are split
    into 4 chunks of 26; replica rows are ordered (chunk, core, window, pos).
    Each hop's AllGather is split into 4 chunk collectives fired as soon as
    that chunk's windows drain, overlapping the collective with the remaining
    supers' gathers/matmuls instead of serializing at the hop boundary.
  - Per hop: dma_gather pulls per-edge 256B rows (4 SWDGE queues); PE computes
    the segment-sum via one-hot indicator matmuls (built on DVE) accumulating
    in fp32 PSUM; Act drains PSUM with the deg^-1 scale into an fp32 buffer
    and batch-converts each chunk to bf16 for the collective.
  - Gating is fused into the hop loop: per chunk, z = <g, s> (DVE reduce),
    sigma = sigmoid(z * sqrt(deg)) (Act), acc += sigma * g (DVE). No final
    re-read phase; output = sqrt(deg) * acc.
"""

import sys

sys.path.insert(0, "/opt/trn_rl_repo")

import numpy as np
import ml_dtypes

FP16 = ml_dtypes.bfloat16


# ----------------------------------------------------------------------------
# Problem constants (hardcoded per spec nn_DAGNNConv_1846835938000).
# ----------------------------------------------------------------------------
def _config(n_nodes, k_hops, n_cores, w_per_core, w_per_super, t_per_bucket):
    g = globals()
    g["N_NODES"] = n_nodes
    g["D"] = 64
    g["K_HOPS"] = k_hops
    g["N_CORES"] = n_cores
    g["WIN"] = 128
    g["W_PER_CORE"] = w_per_core
    assert w_per_core * n_cores * 128 >= n_nodes
    g["ROWS_PC"] = w_per_core * 128
    g["REP_ROWS"] = n_cores * g["ROWS_PC"]
    g["N_SRC_WIN"] = 4
    assert w_per_core % 4 == 0
    g["CHUNK_W"] = w_per_core // 4          # windows per collective chunk
    g["CH_LOC"] = g["CHUNK_W"] * 128        # local rows per chunk
    g["SRC_WIN"] = n_cores * g["CH_LOC"]    # replica rows per chunk
    assert g["SRC_WIN"] <= 32768
    # chunk 3 is split into two half-chunks (G6/G7) with their own layout
    # regions and collectives, so the hop-boundary collective is half-sized.
    assert g["CHUNK_W"] % 2 == 0
    g["QG_W"] = g["CHUNK_W"] // 2
    g["QG_LOC"] = g["QG_W"] * 128
    g["G6_BASE"] = 3 * g["SRC_WIN"]
    g["G7_BASE"] = 3 * g["SRC_WIN"] + n_cores * g["QG_LOC"]
    g["W_PER_SUPER"] = w_per_super
    assert w_per_core % w_per_super == 0
    assert w_per_super <= 8
    g["SUPERS"] = w_per_core // w_per_super
    g["T_PER_BUCKET"] = t_per_bucket
    g["SLOTS_PER_WS"] = t_per_bucket * 128
    g["BUCKET_SLOTS"] = w_per_super * g["SLOTS_PER_WS"]
    g["SLOTS_TOTAL"] = w_per_core * 4 * g["SLOTS_PER_WS"]
    g["TILES_TOTAL"] = g["SLOTS_TOTAL"] // 128
    g["WT"] = w_per_super * t_per_bucket


_config(100000, 10, 8, 104, 8, 3)


def _rep_row_of(core_of, wloc_of, pos_of):
    """Replica row: (chunk, core, window-within-chunk, pos) major order;
    chunk 3 is two half-chunk regions (G6, G7) with the same structure."""
    q = wloc_of // CHUNK_W
    in_q3 = wloc_of >= 3 * CHUNK_W
    half = (wloc_of >= 3 * CHUNK_W + QG_W).astype(np.int64)
    base = np.where(
        in_q3,
        3 * SRC_WIN + half * (N_CORES * QG_LOC),
        q.astype(np.int64) * SRC_WIN,
    )
    wrel = np.where(
        in_q3,
        wloc_of - (3 * CHUNK_W + half * QG_W),
        wloc_of - q.astype(np.int64) * CHUNK_W,
    )
    loc = np.where(in_q3, QG_LOC, CH_LOC)
    return (
        base
        + core_of.astype(np.int64) * loc
        + wrel.astype(np.int64) * 128
        + pos_of.astype(np.int64)
    )


# ----------------------------------------------------------------------------
# Host preprocessing
# ----------------------------------------------------------------------------
def _balance_assign(deg_s_fn, tot):
    """Assign nodes to global windows (N_CORES*W_PER_CORE, cap 128 each) so
    that every (window, src-chunk) edge count stays <= SLOTS_PER_WS."""
    import heapq

    n = tot.shape[0]
    n_windows = N_CORES * W_PER_CORE
    order = np.argsort(-tot, kind="stable")
    heap = [(0, w) for w in range(n_windows)]
    heapq.heapify(heap)
    win_of = np.empty(n, dtype=np.int32)
    win_fill = np.zeros(n_windows, dtype=np.int32)
    for v in order:
        while True:
            load, w = heapq.heappop(heap)
            if win_fill[w] < WIN:
                break
        win_of[v] = w
        win_fill[w] += 1
        if win_fill[w] < WIN:
            heapq.heappush(heap, (load + int(tot[v]), w))

    rng = np.random.default_rng(12345)
    cap = SLOTS_PER_WS
    for round_i in range(12):
        pos_of = np.zeros(n, dtype=np.int32)
        ordv = np.lexsort((np.arange(n), win_of))
        posctr = np.zeros(n_windows, dtype=np.int32)
        for v in ordv:
            pos_of[v] = posctr[win_of[v]]
            posctr[win_of[v]] += 1
        core_of = (win_of // W_PER_CORE).astype(np.int32)
        wloc_of = (win_of % W_PER_CORE).astype(np.int32)
        deg_s = deg_s_fn(core_of, wloc_of, pos_of)  # [n, 4]
        loads = np.zeros((n_windows, N_SRC_WIN), dtype=np.int64)
        np.add.at(loads, win_of, deg_s)
        over = np.flatnonzero((loads > cap).any(axis=1))
        if len(over) == 0:
            return core_of, wloc_of, pos_of
        for w in over:
            s_bad = int(np.argmax(loads[w]))
            excess = int(loads[w, s_bad] - cap)
            members = np.flatnonzero(win_of == w)
            mdeg = deg_s[members, s_bad]
            for v in members[np.argsort(-mdeg)]:
                if excess <= 0:
                    break
                cands = rng.integers(0, n_windows, 64)
                best, bestval = -1, None
                for cw in cands:
                    if cw == w or posctr[cw] >= WIN:
                        continue
                    val = int((loads[cw] + deg_s[v]).max())
                    if val <= cap - 8 and (bestval is None or val < bestval):
                        best, bestval = int(cw), val
                if best < 0:
                    continue
                loads[w] -= deg_s[v]
                loads[best] += deg_s[v]
                win_of[v] = best
                posctr[w] -= 1
                posctr[best] += 1
                excess -= int(deg_s[v, s_bad])
    raise RuntimeError("balance repair failed to converge")


def _preprocess(feats, s, src, dst):
    src = np.asarray(src, dtype=np.int64)
    dst = np.asarray(dst, dtype=np.int64)
    n = N_NODES
    deg = np.bincount(dst, minlength=n).astype(np.float64)
    norm = (deg ** -0.5).astype(np.float32)
    n2 = (1.0 / deg).astype(np.float32)
    sqrtdeg = np.sqrt(deg).astype(np.float32)

    # ---- peel one self-loop per node (handled via identity matmul) ----
    loop_mask = src == dst
    loop_idx = np.flatnonzero(loop_mask)
    uniq_nodes, first_pos = np.unique(dst[loop_idx], return_index=True)
    if len(uniq_nodes) != n:
        raise RuntimeError("not every node has a self-loop; identity fold invalid")
    drop = np.zeros(len(src), dtype=bool)
    drop[loop_idx[first_pos]] = True
    src = src[~drop]
    dst = dst[~drop]

    # ---- node assignment (core, window, pos) ----
    deg_r = np.bincount(dst, minlength=n).astype(np.int64)

    def deg_s_fn(core_of, wloc_of, pos_of):
        rep_row = _rep_row_of(core_of, wloc_of, pos_of)
        es = rep_row[src] // SRC_WIN
        out = np.zeros((n, N_SRC_WIN), dtype=np.int64)
        np.add.at(out, (dst, es), 1)
        return out

    core_of, wloc_of, pos_of = _balance_assign(deg_s_fn, deg_r)
    rep_row = _rep_row_of(core_of, wloc_of, pos_of)

    # ---- per-core edge bucketing ----
    e_core = core_of[dst]
    e_w = wloc_of[dst]            # window of dst within core
    e_key = pos_of[dst]           # indicator column = position of dst in window
    e_srow = rep_row[src]         # replica row of src
    e_s = e_srow // SRC_WIN       # src chunk id (0..3)
    e_gidx = (e_srow - e_s * SRC_WIN).astype(np.int64)  # int16-safe

    gidx_all = np.zeros((N_CORES, SLOTS_TOTAL), dtype=np.int16)
    keys_all = np.full((N_CORES, SLOTS_TOTAL), -1.0, dtype=FP16)

    for c in range(N_CORES):
        m = e_core == c
        cw = e_w[m]
        cs = e_s[m]
        ckey = e_key[m]
        cg = e_gidx[m]
        ws = cw * N_SRC_WIN + cs
        order = np.argsort(ws, kind="stable")
        cw, cs, ckey, cg, ws = cw[order], cs[order], ckey[order], cg[order], ws[order]
        counts = np.bincount(ws, minlength=W_PER_CORE * N_SRC_WIN)
        if counts.max() > SLOTS_PER_WS:
            raise RuntimeError(f"bucket overflow: {counts.max()} > {SLOTS_PER_WS}")
        w_arr = np.arange(W_PER_CORE * N_SRC_WIN) // N_SRC_WIN
        s_arr = np.arange(W_PER_CORE * N_SRC_WIN) % N_SRC_WIN
        starts = (
            ((w_arr // W_PER_SUPER) * N_SRC_WIN + s_arr) * BUCKET_SLOTS
            + (w_arr % W_PER_SUPER) * SLOTS_PER_WS
        )
        runpos = np.arange(len(ws)) - np.repeat(
            np.concatenate([[0], np.cumsum(counts)[:-1]]), counts
        )
        slots = starts[ws] + runpos
        gidx_all[c, slots] = cg.astype(np.int16)
        keys_all[c, slots] = ckey.astype(FP16)

    # ---- initial g0 = norm * feats, local w-major rows (w*128 + p) ----
    g0 = feats.astype(np.float32) * norm[:, None]
    g0_loc = np.zeros((N_CORES, ROWS_PC, 2 * D), dtype=FP16)
    loc_row = wloc_of.astype(np.int64) * 128 + pos_of.astype(np.int64)
    g0_loc[core_of, loc_row, :D] = g0.astype(FP16)

    def pw_table(vec):  # vec [N] -> [N_CORES, 128, W_PER_CORE]
        out = np.zeros((N_CORES, WIN, W_PER_CORE), dtype=np.float32)
        out[core_of, pos_of, wloc_of] = vec
        return out

    n2_pw = pw_table(n2)
    n2_pw[n2_pw == 0] = 1.0
    sqd_pw = pw_table(sqrtdeg)

    gidx_wrapped = np.ascontiguousarray(
        np.tile(
            gidx_all.reshape(N_CORES, SLOTS_TOTAL // 16, 16).transpose(0, 2, 1),
            (1, 8, 1),
        )
    )
    keys_tiles = np.ascontiguousarray(
        keys_all.reshape(N_CORES, TILES_TOTAL, 128).transpose(0, 2, 1)
    )

    iota = np.broadcast_to(np.arange(128, dtype=np.float32), (128, 128)).astype(FP16)
    iota = np.ascontiguousarray(iota)
    s_bcast = np.broadcast_to(np.asarray(s, dtype=np.float32).reshape(1, D), (128, D))
    s_bcast = np.ascontiguousarray(s_bcast)

    in_maps = []
    for c in range(N_CORES):
        in_maps.append(
            {
                "g0_own": np.ascontiguousarray(g0_loc[c]),
                "gidx": gidx_wrapped[c],
                "keys": keys_tiles[c],
                "n2_pw": np.ascontiguousarray(n2_pw[c]),
                "sqd_pw": np.ascontiguousarray(sqd_pw[c]),
                "s_bcast": s_bcast,
                "iota": iota,
            }
        )
    meta = {"core_of": core_of, "wloc_of": wloc_of, "pos_of": pos_of}
    return in_maps, meta


# ----------------------------------------------------------------------------
# Bass kernel builder (identical program for all cores)
# ----------------------------------------------------------------------------
def _build():
    import concourse.bacc as bacc
    import concourse.mybir as mybir
    from concourse.tile import TileContext

    fp32 = mybir.dt.float32
    fp16 = mybir.dt.bfloat16
    i16 = mybir.dt.int16

    nc = bacc.Bacc(None, target_bir_lowering=False, num_devices=N_CORES, num_swdge_queues=4)

    # I/O
    g0_own = nc.dram_tensor("g0_own", [ROWS_PC, 2 * D], fp16, kind="ExternalInput")
    gidx_in = nc.dram_tensor("gidx", [128, SLOTS_TOTAL // 16], i16, kind="ExternalInput")
    keys_in = nc.dram_tensor("keys", [128, TILES_TOTAL], fp16, kind="ExternalInput")
    n2_in = nc.dram_tensor("n2_pw", [128, W_PER_CORE], fp32, kind="ExternalInput")
    sqd_in = nc.dram_tensor("sqd_pw", [128, W_PER_CORE], fp32, kind="ExternalInput")
    s_in = nc.dram_tensor("s_bcast", [128, D], fp32, kind="ExternalInput")
    iota_in = nc.dram_tensor("iota", [128, 128], fp16, kind="ExternalInput")
    out_pm = nc.dram_tensor("out_pm", [ROWS_PC, D], fp32, kind="ExternalOutput")

    # hop buffers: cc_in[h] local slice for hop h (h=0 is the bootstrap copy),
    # cc_out[h] the shared replica gathered from it.
    cc_in = [
        nc.dram_tensor(f"cc_in_{h}", [ROWS_PC, 2 * D], fp16) for h in range(K_HOPS)
    ]
    cc_out = [
        nc.dram_tensor(f"cc_out_{h}", [REP_ROWS, 2 * D], fp16, addr_space="Shared")
        for h in range(K_HOPS)
    ]
    groups = [list(range(N_CORES))]

    # Collectives must run on gpsimd (walrus checkValidEngines rejects other
    # engines). The cc's SEQ wait head-of-line blocks Pool — descriptor
    # generation stalls — so the gather issue order below keeps >=2 supers of
    # descriptors queued whenever Pool can block on a collective.
    def cc_allgather(in_ap, out_ap):
        nc.gpsimd.collective_compute(
            "AllGather",
            mybir.AluOpType.bypass,
            replica_groups=groups,
            ins=[in_ap],
            outs=[out_ap],
        )

    with TileContext(nc) as tc:
        with tc.tile_pool(name="persist", bufs=1) as pp:
            # ---- static tables ----
            gidx_sb = pp.tile([128, SLOTS_TOTAL // 16], i16, tag="gidx")
            nc.sync.dma_start(out=gidx_sb[:, :], in_=gidx_in[:, :])
            keys_sb = pp.tile([128, TILES_TOTAL], fp16, tag="keys")
            nc.sync.dma_start(out=keys_sb[:, :], in_=keys_in[:, :])
            n2_sb = pp.tile([128, W_PER_CORE], fp32, tag="n2")
            nc.sync.dma_start(out=n2_sb[:, :], in_=n2_in[:, :])
            sqd_sb = pp.tile([128, W_PER_CORE], fp32, tag="sqd")
            nc.sync.dma_start(out=sqd_sb[:, :], in_=sqd_in[:, :])
            s_sb = pp.tile([128, D], fp32, tag="svec")
            nc.sync.dma_start(out=s_sb[:, :], in_=s_in[:, :])
            iota_sb = pp.tile([128, 128], fp16, tag="iota")
            nc.sync.dma_start(out=iota_sb[:, :], in_=iota_in[:, :])
            # tiled iota for the indicator builds: iota_wt[p, j, f] = f
            iota_wt = pp.tile([128, WT, 128], fp16, tag="iota_wt")
            with tc.tile_pool(name="boot_tmp", bufs=1) as btp:
                zero_wt = btp.tile([128, WT, 128], fp16, tag="zero_wt")
                nc.vector.memset(zero_wt[:, :, :], 0.0)
                nc.vector.tensor_tensor(
                    iota_wt[:, :, :],
                    iota_sb[:, :]
                    .rearrange("p (one f) -> p one f", one=1)
                    .broadcast_to((128, WT, 128)),
                    zero_wt[:, :, :],
                    mybir.AluOpType.add,
                )
            # identity (fp16) for the self-loop fold
            ident_sb = pp.tile([128, 128], fp16, tag="ident")
            pidx_sb = pp.tile([128, 1], fp32, tag="pidx")
            nc.gpsimd.iota(
                pidx_sb[:, :],
                [[1, 1]],
                base=0,
                channel_multiplier=1,
                allow_small_or_imprecise_dtypes=True,
            )
            nc.vector.tensor_scalar(
                ident_sb[:, :],
                iota_sb[:, :],
                pidx_sb[:, :],
                None,
                mybir.AluOpType.is_equal,
            )

            # ---- persistent state ----
            staged = pp.tile([128, W_PER_CORE, 2 * D], fp16, tag="staged")
            g32 = pp.tile([128, W_PER_CORE, D], fp32, tag="g32")
            acc = pp.tile([128, W_PER_CORE, D], fp32, tag="acc")
            tmp = pp.tile([128, CHUNK_W, D], fp32, tag="tmp")
            zt = pp.tile([128, W_PER_CORE], fp32, tag="zt")
            sig = pp.tile([128, W_PER_CORE], fp32, tag="sig")
            nc.vector.memset(acc[:, :, :], 0.0)

            def gating_chunk(q):
                """acc[chunk q] += sigmoid(<g32,s> * sqd) * g32[chunk q]."""
                c0 = q * CHUNK_W
                sl = slice(c0, c0 + CHUNK_W)
                nc.vector.tensor_tensor(
                    tmp[:, :, :],
                    g32[:, sl, :],
                    s_sb[:, :]
                    .rearrange("p (one f) -> p one f", one=1)
                    .broadcast_to((128, CHUNK_W, D)),
                    mybir.AluOpType.mult,
                )
                nc.vector.tensor_reduce(
                    zt[:, sl],
                    tmp[:, :, :],
                    mybir.AxisListType.X,
                    mybir.AluOpType.add,
                )
                nc.vector.tensor_tensor(
                    sig[:, sl], zt[:, sl], sqd_sb[:, sl], mybir.AluOpType.mult
                )
                nc.scalar.activation(
                    sig[:, sl], sig[:, sl], mybir.ActivationFunctionType.Sigmoid
                )
                nc.vector.tensor_tensor(
                    tmp[:, :, :],
                    g32[:, sl, :],
                    sig[:, sl]
                    .rearrange("p (w one) -> p w one", one=1)
                    .broadcast_to((128, CHUNK_W, D)),
                    mybir.AluOpType.mult,
                )
                nc.vector.tensor_tensor(
                    acc[:, sl, :], tmp[:, :, :], acc[:, sl, :], mybir.AluOpType.add
                )

            # ---- bootstrap: staged/g32 = g0; chunked AllGather -> cc_out[0] ----
            nc.sync.dma_start(out=cc_in[0][:, :], in_=g0_own[:, :])
            nc.sync.dma_start(
                out=staged[:, :, :],
                in_=g0_own[:, :].rearrange("(w p) f -> p w f", p=128),
            )
            def fire_ccs(h, w):
                """Convert+stage+AllGather the layout region ending at window
                w (w is the last window of a chunk or half-chunk)."""
                if w == CHUNK_W - 1 or w == 2 * CHUNK_W - 1 or w == 3 * CHUNK_W - 1:
                    q = w // CHUNK_W
                    w0, nw = q * CHUNK_W, CHUNK_W
                    out0, outn = q * SRC_WIN, SRC_WIN
                elif w == 3 * CHUNK_W + QG_W - 1:
                    w0, nw = 3 * CHUNK_W, QG_W
                    out0, outn = G6_BASE, N_CORES * QG_LOC
                elif w == W_PER_CORE - 1:
                    w0, nw = 3 * CHUNK_W + QG_W, QG_W
                    out0, outn = G7_BASE, N_CORES * QG_LOC
                else:
                    return
                nc.scalar.activation(
                    staged[:, w0 : w0 + nw, 0:D],
                    g32[:, w0 : w0 + nw, :],
                    mybir.ActivationFunctionType.Copy,
                )
                nc.sync.dma_start(
                    out=cc_in[h][w0 * 128 : (w0 + nw) * 128, :].rearrange(
                        "(w p) f -> p w f", p=128
                    ),
                    in_=staged[:, w0 : w0 + nw, :],
                )
                cc_allgather(
                    cc_in[h][w0 * 128 : (w0 + nw) * 128, :],
                    cc_out[h][out0 : out0 + outn, :],
                )

            for q in range(N_SRC_WIN - 1):
                cc_allgather(
                    cc_in[0][q * CH_LOC : (q + 1) * CH_LOC, :],
                    cc_out[0][q * SRC_WIN : (q + 1) * SRC_WIN, :],
                )
            cc_allgather(
                cc_in[0][3 * CH_LOC : 3 * CH_LOC + QG_LOC, :],
                cc_out[0][G6_BASE : G6_BASE + N_CORES * QG_LOC, :],
            )
            cc_allgather(
                cc_in[0][3 * CH_LOC + QG_LOC : ROWS_PC, :],
                cc_out[0][G7_BASE : G7_BASE + N_CORES * QG_LOC, :],
            )
            nc.scalar.activation(
                g32[:, :, :],
                staged[:, :, 0:D],
                mybir.ActivationFunctionType.Copy,
            )
            for q in range(N_SRC_WIN):
                gating_chunk(q)

            with (
                tc.tile_pool(name="chunks", bufs=13) as chunk_pool,
                tc.tile_pool(name="inds", bufs=1) as ind_pool,
                tc.tile_pool(name="psum", bufs=8, space="PSUM") as psum_pool,
            ):
                for h in range(1, K_HOPS + 1):
                    src_rep = cc_out[h - 1]
                    pending = {}

                    def issue_gather(sup, s):
                        ch = chunk_pool.tile(
                            [128, BUCKET_SLOTS // 128, 2 * D], fp16, tag="chunk"
                        )
                        bucket = sup * N_SRC_WIN + s
                        col0 = bucket * (BUCKET_SLOTS // 16)
                        row0 = s * SRC_WIN
                        nc.gpsimd.dma_gather(
                            ch[:, :, :],
                            src_rep[row0 : row0 + SRC_WIN, :],
                            gidx_sb[:, col0 : col0 + BUCKET_SLOTS // 16],
                            BUCKET_SLOTS,
                            BUCKET_SLOTS,
                            2 * D,
                            single_packet=False,
                            queue_num=(s + sup) % N_SRC_WIN,
                        )
                        pending[(sup, s)] = ch

                    # Hop prologue: issue the first two supers' gathers with
                    # the chunk-3 reads LAST, so the Pool sequencer has ~2
                    # supers of dependency-free descriptor generation before
                    # it head-of-line blocks on the previous hop's chunk-3
                    # collective.
                    for sup in range(min(2, SUPERS)):
                        for s in range(N_SRC_WIN - 1):
                            issue_gather(sup, s)
                    for sup in range(min(2, SUPERS)):
                        issue_gather(sup, N_SRC_WIN - 1)

                    for sup in range(SUPERS):
                        if sup + 2 < SUPERS:
                            for s in range(N_SRC_WIN):
                                issue_gather(sup + 2, s)
                        chunks = [pending.pop((sup, s)) for s in range(N_SRC_WIN)]
                        banks = [
                            psum_pool.tile([128, D], fp32, tag="bank", name="bank")
                            for _ in range(W_PER_SUPER)
                        ]
                        for s in range(N_SRC_WIN):
                            col0 = (sup * N_SRC_WIN + s) * WT
                            indb = ind_pool.tile([128, WT, 128], fp16, tag="ind")
                            nc.vector.tensor_tensor(
                                indb[:, :, :],
                                iota_wt[:, :, :],
                                keys_sb[:, col0 : col0 + WT].broadcast_to(
                                    (128, WT, 128)
                                ),
                                mybir.AluOpType.is_equal,
                            )
                            for wi in range(W_PER_SUPER):
                                w = sup * W_PER_SUPER + wi
                                bank = banks[wi]
                                for t in range(T_PER_BUCKET):
                                    nc.tensor.matmul(
                                        bank[:, :],
                                        indb[:, wi * T_PER_BUCKET + t, :],
                                        chunks[s][:, wi * T_PER_BUCKET + t, 0:D],
                                        start=(s == 0 and t == 0),
                                        stop=False,
                                    )
                                if s == N_SRC_WIN - 1:
                                    nc.tensor.matmul(
                                        bank[:, :],
                                        ident_sb[:, :],
                                        staged[:, w, 0:D],
                                        start=False,
                                        stop=True,
                                    )
                        for wi in range(W_PER_SUPER):
                            w = sup * W_PER_SUPER + wi
                            nc.scalar.activation(
                                g32[:, w, :],
                                banks[wi][:, :],
                                mybir.ActivationFunctionType.Copy,
                                scale=n2_sb[:, w : w + 1],
                            )
                            if h < K_HOPS:
                                fire_ccs(h, w)
                            if w % CHUNK_W == CHUNK_W - 1:
                                gating_chunk(w // CHUNK_W)

            # ---- output: out = sqd * acc (reuse g32 as fp32 staging) ----
            nc.vector.tensor_tensor(
                g32[:, :, :],
                acc[:, :, :],
                sqd_sb[:, :]
                .rearrange("p (w one) -> p w one", one=1)
                .broadcast_to((128, W_PER_CORE, D)),
                mybir.AluOpType.mult,
            )
            nc.sync.dma_start(
                out=out_pm[:, :].rearrange("(w p) f -> p w f", p=128),
                in_=g32[:, :, :],
            )

    nc.finalize()
    return nc


# ----------------------------------------------------------------------------
# Entry point
# ----------------------------------------------------------------------------
_CACHED = {}


def kernel(**inputs):
    feats = np.asarray(inputs["feats"], dtype=np.float32)
    s = np.asarray(inputs["s"], dtype=np.float32)
    src = np.asarray(inputs["src"])
    dst = np.asarray(inputs["dst"])

    in_maps, meta = _preprocess(feats, s, src, dst)

    from concourse.bass_utils import run_bass_kernel_spmd

    nc = _CACHED.get("nc")
    if nc is None:
        nc = _build()
        _CACHED["nc"] = nc

    res = run_bass_kernel_spmd(nc, in_maps, core_ids=list(range(N_CORES)))
    _CACHED["last_result"] = res
    # unshard: out_pm row for node at (core, window w, pos p) is w*128 + p
    out = np.zeros((N_NODES, D), dtype=np.float32)
    core_of, wloc_of, pos_of = meta["core_of"], meta["wloc_of"], meta["pos_of"]
    rows = wloc_of.astype(np.int64) * 128 + pos_of.astype(np.int64)
    for c in range(N_CORES):
        m = core_of == c
        out[m] = res.results[c]["out_pm"][rows[m]]
    return out


if __name__ == "__main__":
    nc = _build()
    print("build ok")


# revision 29
# speedup vs baseline: 1.0624x; 1.0624x over previous
"""DAGNN (10-hop propagation + sigmoid gating) Bass kernel for 8 trn2 NeuronCores.

Strategy (1D node partition, SPMD-uniform schedule), v2:
  - Host assigns nodes to (core, window, slot) with degree balancing so every
    core runs an identical instruction stream (one NEFF, 8 cores).
  - Replica layout is WINDOW-CHUNK major: the 104 windows per core are split
    into 4 chunks of 26; replica rows are ordered (chunk, core, window, pos).
    Each hop's AllGather is split into 4 chunk collectives fired as soon as
    that chunk's windows drain, overlapping the collective with the remaining
    supers' gathers/matmuls instead of serializing at the hop boundary.
  - Per hop: dma_gather pulls per-edge 256B rows (4 SWDGE queues); PE computes
    the segment-sum via one-hot indicator matmuls (built on DVE) accumulating
    in fp32 PSUM; Act drains PSUM with the deg^-1 scale into an fp32 buffer
    and batch-converts each chunk to bf16 for the collective.
  - Gating is fused into the hop loop: per chunk, z = <g, s> (DVE reduce),
    sigma = sigmoid(z * sqrt(deg)) (Act), acc += sigma * g (DVE). No final
    re-read phase; output = sqrt(deg) * acc.
"""

import sys

sys.path.insert(0, "/opt/trn_rl_repo")

import numpy as np
import ml_dtypes

FP16 = ml_dtypes.bfloat16


# ----------------------------------------------------------------------------
# Problem constants (hardcoded per spec nn_DAGNNConv_1846835938000).
# ----------------------------------------------------------------------------
def _config(n_nodes, k_hops, n_cores, w_per_core, w_per_super, t_per_bucket):
    g = globals()
    g["N_NODES"] = n_nodes
    g["D"] = 64
    g["K_HOPS"] = k_hops
    g["N_CORES"] = n_cores
    g["WIN"] = 128
    g["W_PER_CORE"] = w_per_core
    assert w_per_core * n_cores * 128 >= n_nodes
    g["ROWS_PC"] = w_per_core * 128
    g["REP_ROWS"] = n_cores * g["ROWS_PC"]
    g["N_SRC_WIN"] = 4
    assert w_per_core % 4 == 0
    g["CHUNK_W"] = w_per_core // 4          # windows per collective chunk
    g["CH_LOC"] = g["CHUNK_W"] * 128        # local rows per chunk
    g["SRC_WIN"] = n_cores * g["CH_LOC"]    # replica rows per chunk
    assert g["SRC_WIN"] <= 32768
    g["W_PER_SUPER"] = w_per_super
    assert w_per_core % w_per_super == 0
    assert w_per_super <= 8
    g["SUPERS"] = w_per_core // w_per_super
    g["T_PER_BUCKET"] = t_per_bucket
    g["SLOTS_PER_WS"] = t_per_bucket * 128
    g["BUCKET_SLOTS"] = w_per_super * g["SLOTS_PER_WS"]
    g["SLOTS_TOTAL"] = w_per_core * 4 * g["SLOTS_PER_WS"]
    g["TILES_TOTAL"] = g["SLOTS_TOTAL"] // 128
    g["WT"] = w_per_super * t_per_bucket


_config(100000, 10, 8, 104, 8, 3)


def _rep_row_of(core_of, wloc_of, pos_of):
    """Replica row: (chunk, core, window-within-chunk, pos) major order."""
    q = wloc_of // CHUNK_W
    return (
        q.astype(np.int64) * SRC_WIN
        + core_of.astype(np.int64) * CH_LOC
        + (wloc_of - q * CHUNK_W).astype(np.int64) * 128
        + pos_of.astype(np.int64)
    )


# ----------------------------------------------------------------------------
# Host preprocessing
# ----------------------------------------------------------------------------
def _balance_assign(deg_s_fn, tot):
    """Assign nodes to global windows (N_CORES*W_PER_CORE, cap 128 each) so
    that every (window, src-chunk) edge count stays <= SLOTS_PER_WS."""
    import heapq

    n = tot.shape[0]
    n_windows = N_CORES * W_PER_CORE
    order = np.argsort(-tot, kind="stable")
    heap = [(0, w) for w in range(n_windows)]
    heapq.heapify(heap)
    win_of = np.empty(n, dtype=np.int32)
    win_fill = np.zeros(n_windows, dtype=np.int32)
    for v in order:
        while True:
            load, w = heapq.heappop(heap)
            if win_fill[w] < WIN:
                break
        win_of[v] = w
        win_fill[w] += 1
        if win_fill[w] < WIN:
            heapq.heappush(heap, (load + int(tot[v]), w))

    rng = np.random.default_rng(12345)
    cap = SLOTS_PER_WS
    for round_i in range(12):
        pos_of = np.zeros(n, dtype=np.int32)
        ordv = np.lexsort((np.arange(n), win_of))
        posctr = np.zeros(n_windows, dtype=np.int32)
        for v in ordv:
            pos_of[v] = posctr[win_of[v]]
            posctr[win_of[v]] += 1
        core_of = (win_of // W_PER_CORE).astype(np.int32)
        wloc_of = (win_of % W_PER_CORE).astype(np.int32)
        deg_s = deg_s_fn(core_of, wloc_of, pos_of)  # [n, 4]
        loads = np.zeros((n_windows, N_SRC_WIN), dtype=np.int64)
        np.add.at(loads, win_of, deg_s)
        over = np.flatnonzero((loads > cap).any(axis=1))
        if len(over) == 0:
            return core_of, wloc_of, pos_of
        for w in over:
            s_bad = int(np.argmax(loads[w]))
            excess = int(loads[w, s_bad] - cap)
            members = np.flatnonzero(win_of == w)
            mdeg = deg_s[members, s_bad]
            for v in members[np.argsort(-mdeg)]:
                if excess <= 0:
                    break
                cands = rng.integers(0, n_windows, 64)
                best, bestval = -1, None
                for cw in cands:
                    if cw == w or posctr[cw] >= WIN:
                        continue
                    val = int((loads[cw] + deg_s[v]).max())
                    if val <= cap - 8 and (bestval is None or val < bestval):
                        best, bestval = int(cw), val
                if best < 0:
                    continue
                loads[w] -= deg_s[v]
                loads[best] += deg_s[v]
                win_of[v] = best
                posctr[w] -= 1
                posctr[best] += 1
                excess -= int(deg_s[v, s_bad])
    raise RuntimeError("balance repair failed to converge")


def _preprocess(feats, s, src, dst):
    src = np.asarray(src, dtype=np.int64)
    dst = np.asarray(dst, dtype=np.int64)
    n = N_NODES
    deg = np.bincount(dst, minlength=n).astype(np.float64)
    norm = (deg ** -0.5).astype(np.float32)
    n2 = (1.0 / deg).astype(np.float32)
    sqrtdeg = np.sqrt(deg).astype(np.float32)

    # ---- peel one self-loop per node (handled via identity matmul) ----
    loop_mask = src == dst
    loop_idx = np.flatnonzero(loop_mask)
    uniq_nodes, first_pos = np.unique(dst[loop_idx], return_index=True)
    if len(uniq_nodes) != n:
        raise RuntimeError("not every node has a self-loop; identity fold invalid")
    drop = np.zeros(len(src), dtype=bool)
    drop[loop_idx[first_pos]] = True
    src = src[~drop]
    dst = dst[~drop]

    # ---- node assignment (core, window, pos) ----
    deg_r = np.bincount(dst, minlength=n).astype(np.int64)

    def deg_s_fn(core_of, wloc_of, pos_of):
        rep_row = _rep_row_of(core_of, wloc_of, pos_of)
        es = rep_row[src] // SRC_WIN
        out = np.zeros((n, N_SRC_WIN), dtype=np.int64)
        np.add.at(out, (dst, es), 1)
        return out

    core_of, wloc_of, pos_of = _balance_assign(deg_s_fn, deg_r)
    rep_row = _rep_row_of(core_of, wloc_of, pos_of)

    # ---- per-core edge bucketing ----
    e_core = core_of[dst]
    e_w = wloc_of[dst]            # window of dst within core
    e_key = pos_of[dst]           # indicator column = position of dst in window
    e_srow = rep_row[src]         # replica row of src
    e_s = e_srow // SRC_WIN       # src chunk id (0..3)
    e_gidx = (e_srow - e_s * SRC_WIN).astype(np.int64)  # int16-safe

    gidx_all = np.zeros((N_CORES, SLOTS_TOTAL), dtype=np.int16)
    keys_all = np.full((N_CORES, SLOTS_TOTAL), -1.0, dtype=FP16)

    for c in range(N_CORES):
        m = e_core == c
        cw = e_w[m]
        cs = e_s[m]
        ckey = e_key[m]
        cg = e_gidx[m]
        ws = cw * N_SRC_WIN + cs
        order = np.argsort(ws, kind="stable")
        cw, cs, ckey, cg, ws = cw[order], cs[order], ckey[order], cg[order], ws[order]
        counts = np.bincount(ws, minlength=W_PER_CORE * N_SRC_WIN)
        if counts.max() > SLOTS_PER_WS:
            raise RuntimeError(f"bucket overflow: {counts.max()} > {SLOTS_PER_WS}")
        w_arr = np.arange(W_PER_CORE * N_SRC_WIN) // N_SRC_WIN
        s_arr = np.arange(W_PER_CORE * N_SRC_WIN) % N_SRC_WIN
        starts = (
            ((w_arr // W_PER_SUPER) * N_SRC_WIN + s_arr) * BUCKET_SLOTS
            + (w_arr % W_PER_SUPER) * SLOTS_PER_WS
        )
        runpos = np.arange(len(ws)) - np.repeat(
            np.concatenate([[0], np.cumsum(counts)[:-1]]), counts
        )
        slots = starts[ws] + runpos
        gidx_all[c, slots] = cg.astype(np.int16)
        keys_all[c, slots] = ckey.astype(FP16)

    # ---- initial g0 = norm * feats, local w-major rows (w*128 + p) ----
    g0 = feats.astype(np.float32) * norm[:, None]
    g0_loc = np.zeros((N_CORES, ROWS_PC, 2 * D), dtype=FP16)
    loc_row = wloc_of.astype(np.int64) * 128 + pos_of.astype(np.int64)
    g0_loc[core_of, loc_row, :D] = g0.astype(FP16)

    def pw_table(vec):  # vec [N] -> [N_CORES, 128, W_PER_CORE]
        out = np.zeros((N_CORES, WIN, W_PER_CORE), dtype=np.float32)
        out[core_of, pos_of, wloc_of] = vec
        return out

    n2_pw = pw_table(n2)
    n2_pw[n2_pw == 0] = 1.0
    sqd_pw = pw_table(sqrtdeg)

    gidx_wrapped = np.ascontiguousarray(
        np.tile(
            gidx_all.reshape(N_CORES, SLOTS_TOTAL // 16, 16).transpose(0, 2, 1),
            (1, 8, 1),
        )
    )
    keys_tiles = np.ascontiguousarray(
        keys_all.reshape(N_CORES, TILES_TOTAL, 128).transpose(0, 2, 1)
    )

    iota = np.broadcast_to(np.arange(128, dtype=np.float32), (128, 128)).astype(FP16)
    iota = np.ascontiguousarray(iota)
    s_bcast = np.broadcast_to(np.asarray(s, dtype=np.float32).reshape(1, D), (128, D))
    s_bcast = np.ascontiguousarray(s_bcast)

    in_maps = []
    for c in range(N_CORES):
        in_maps.append(
            {
                "g0_own": np.ascontiguousarray(g0_loc[c]),
                "gidx": gidx_wrapped[c],
                "keys": keys_tiles[c],
                "n2_pw": np.ascontiguousarray(n2_pw[c]),
                "sqd_pw": np.ascontiguousarray(sqd_pw[c]),
                "s_bcast": s_bcast,
                "iota": iota,
            }
        )
    meta = {"core_of": core_of, "wloc_of": wloc_of, "pos_of": pos_of}
    return in_maps, meta


# ----------------------------------------------------------------------------
# Bass kernel builder (identical program for all cores)
# ----------------------------------------------------------------------------
def _build():
    import concourse.bacc as bacc
    import concourse.mybir as mybir
    from concourse.tile import TileContext

    fp32 = mybir.dt.float32
    fp16 = mybir.dt.bfloat16
    i16 = mybir.dt.int16

    nc = bacc.Bacc(None, target_bir_lowering=False, num_devices=N_CORES, num_swdge_queues=4)

    # I/O
    g0_own = nc.dram_tensor("g0_own", [ROWS_PC, 2 * D], fp16, kind="ExternalInput")
    gidx_in = nc.dram_tensor("gidx", [128, SLOTS_TOTAL // 16], i16, kind="ExternalInput")
    keys_in = nc.dram_tensor("keys", [128, TILES_TOTAL], fp16, kind="ExternalInput")
    n2_in = nc.dram_tensor("n2_pw", [128, W_PER_CORE], fp32, kind="ExternalInput")
    sqd_in = nc.dram_tensor("sqd_pw", [128, W_PER_CORE], fp32, kind="ExternalInput")
    s_in = nc.dram_tensor("s_bcast", [128, D], fp32, kind="ExternalInput")
    iota_in = nc.dram_tensor("iota", [128, 128], fp16, kind="ExternalInput")
    out_pm = nc.dram_tensor("out_pm", [ROWS_PC, D], fp32, kind="ExternalOutput")

    # hop buffers: cc_in[h] local slice for hop h (h=0 is the bootstrap copy),
    # cc_out[h] the shared replica gathered from it.
    cc_in = [
        nc.dram_tensor(f"cc_in_{h}", [ROWS_PC, 2 * D], fp16) for h in range(K_HOPS)
    ]
    cc_out = [
        nc.dram_tensor(f"cc_out_{h}", [REP_ROWS, 2 * D], fp16, addr_space="Shared")
        for h in range(K_HOPS)
    ]
    groups = [list(range(N_CORES))]

    # Collectives must run on gpsimd (walrus checkValidEngines rejects other
    # engines). The cc's SEQ wait head-of-line blocks Pool — descriptor
    # generation stalls — so the gather issue order below keeps >=2 supers of
    # descriptors queued whenever Pool can block on a collective.
    def cc_allgather(in_ap, out_ap):
        nc.gpsimd.collective_compute(
            "AllGather",
            mybir.AluOpType.bypass,
            replica_groups=groups,
            ins=[in_ap],
            outs=[out_ap],
        )

    with TileContext(nc) as tc:
        with tc.tile_pool(name="persist", bufs=1) as pp:
            # ---- static tables ----
            gidx_sb = pp.tile([128, SLOTS_TOTAL // 16], i16, tag="gidx")
            nc.sync.dma_start(out=gidx_sb[:, :], in_=gidx_in[:, :])
            keys_sb = pp.tile([128, TILES_TOTAL], fp16, tag="keys")
            nc.sync.dma_start(out=keys_sb[:, :], in_=keys_in[:, :])
            n2_sb = pp.tile([128, W_PER_CORE], fp32, tag="n2")
            nc.sync.dma_start(out=n2_sb[:, :], in_=n2_in[:, :])
            sqd_sb = pp.tile([128, W_PER_CORE], fp32, tag="sqd")
            nc.sync.dma_start(out=sqd_sb[:, :], in_=sqd_in[:, :])
            s_sb = pp.tile([128, D], fp32, tag="svec")
            nc.sync.dma_start(out=s_sb[:, :], in_=s_in[:, :])
            iota_sb = pp.tile([128, 128], fp16, tag="iota")
            nc.sync.dma_start(out=iota_sb[:, :], in_=iota_in[:, :])
            # tiled iota for the indicator builds: iota_wt[p, j, f] = f
            iota_wt = pp.tile([128, WT, 128], fp16, tag="iota_wt")
            with tc.tile_pool(name="boot_tmp", bufs=1) as btp:
                zero_wt = btp.tile([128, WT, 128], fp16, tag="zero_wt")
                nc.vector.memset(zero_wt[:, :, :], 0.0)
                nc.vector.tensor_tensor(
                    iota_wt[:, :, :],
                    iota_sb[:, :]
                    .rearrange("p (one f) -> p one f", one=1)
                    .broadcast_to((128, WT, 128)),
                    zero_wt[:, :, :],
                    mybir.AluOpType.add,
                )
            # identity (fp16) for the self-loop fold
            ident_sb = pp.tile([128, 128], fp16, tag="ident")
            pidx_sb = pp.tile([128, 1], fp32, tag="pidx")
            nc.gpsimd.iota(
                pidx_sb[:, :],
                [[1, 1]],
                base=0,
                channel_multiplier=1,
                allow_small_or_imprecise_dtypes=True,
            )
            nc.vector.tensor_scalar(
                ident_sb[:, :],
                iota_sb[:, :],
                pidx_sb[:, :],
                None,
                mybir.AluOpType.is_equal,
            )

            # ---- persistent state ----
            staged = pp.tile([128, W_PER_CORE, 2 * D], fp16, tag="staged")
            g32 = pp.tile([128, W_PER_CORE, D], fp32, tag="g32")
            acc = pp.tile([128, W_PER_CORE, D], fp32, tag="acc")
            tmp = pp.tile([128, CHUNK_W, D], fp32, tag="tmp")
            zt = pp.tile([128, W_PER_CORE], fp32, tag="zt")
            sig = pp.tile([128, W_PER_CORE], fp32, tag="sig")
            nc.vector.memset(acc[:, :, :], 0.0)

            def gating_chunk(q):
                """acc[chunk q] += sigmoid(<g32,s> * sqd) * g32[chunk q]."""
                c0 = q * CHUNK_W
                sl = slice(c0, c0 + CHUNK_W)
                nc.vector.tensor_tensor(
                    tmp[:, :, :],
                    g32[:, sl, :],
                    s_sb[:, :]
                    .rearrange("p (one f) -> p one f", one=1)
                    .broadcast_to((128, CHUNK_W, D)),
                    mybir.AluOpType.mult,
                )
                nc.vector.tensor_reduce(
                    zt[:, sl],
                    tmp[:, :, :],
                    mybir.AxisListType.X,
                    mybir.AluOpType.add,
                )
                nc.vector.tensor_tensor(
                    sig[:, sl], zt[:, sl], sqd_sb[:, sl], mybir.AluOpType.mult
                )
                nc.scalar.activation(
                    sig[:, sl], sig[:, sl], mybir.ActivationFunctionType.Sigmoid
                )
                nc.vector.tensor_tensor(
                    tmp[:, :, :],
                    g32[:, sl, :],
                    sig[:, sl]
                    .rearrange("p (w one) -> p w one", one=1)
                    .broadcast_to((128, CHUNK_W, D)),
                    mybir.AluOpType.mult,
                )
                nc.vector.tensor_tensor(
                    acc[:, sl, :], tmp[:, :, :], acc[:, sl, :], mybir.AluOpType.add
                )

            # ---- bootstrap: staged/g32 = g0; chunked AllGather -> cc_out[0] ----
            nc.sync.dma_start(out=cc_in[0][:, :], in_=g0_own[:, :])
            nc.sync.dma_start(
                out=staged[:, :, :],
                in_=g0_own[:, :].rearrange("(w p) f -> p w f", p=128),
            )
            for q in range(N_SRC_WIN):
                cc_allgather(
                    cc_in[0][q * CH_LOC : (q + 1) * CH_LOC, :],
                    cc_out[0][q * SRC_WIN : (q + 1) * SRC_WIN, :],
                )
            nc.scalar.activation(
                g32[:, :, :],
                staged[:, :, 0:D],
                mybir.ActivationFunctionType.Copy,
            )
            for q in range(N_SRC_WIN):
                gating_chunk(q)

            with (
                tc.tile_pool(name="chunks", bufs=12) as chunk_pool,
                tc.tile_pool(name="inds", bufs=2) as ind_pool,
                tc.tile_pool(name="psum", bufs=8, space="PSUM") as psum_pool,
            ):
                for h in range(1, K_HOPS + 1):
                    src_rep = cc_out[h - 1]
                    pending = {}

                    def issue_gather(sup, s):
                        ch = chunk_pool.tile(
                            [128, BUCKET_SLOTS // 128, 2 * D], fp16, tag="chunk"
                        )
                        bucket = sup * N_SRC_WIN + s
                        col0 = bucket * (BUCKET_SLOTS // 16)
                        row0 = s * SRC_WIN
                        nc.gpsimd.dma_gather(
                            ch[:, :, :],
                            src_rep[row0 : row0 + SRC_WIN, :],
                            gidx_sb[:, col0 : col0 + BUCKET_SLOTS // 16],
                            BUCKET_SLOTS,
                            BUCKET_SLOTS,
                            2 * D,
                            single_packet=False,
                            queue_num=(s + sup) % N_SRC_WIN,
                        )
                        pending[(sup, s)] = ch

                    # Hop prologue: issue the first two supers' gathers with
                    # the chunk-3 reads LAST, so the Pool sequencer has ~2
                    # supers of dependency-free descriptor generation before
                    # it head-of-line blocks on the previous hop's chunk-3
                    # collective.
                    for sup in range(min(2, SUPERS)):
                        for s in range(N_SRC_WIN - 1):
                            issue_gather(sup, s)
                    for sup in range(min(2, SUPERS)):
                        issue_gather(sup, N_SRC_WIN - 1)

                    for sup in range(SUPERS):
                        if sup + 2 < SUPERS:
                            for s in range(N_SRC_WIN):
                                issue_gather(sup + 2, s)
                        chunks = [pending.pop((sup, s)) for s in range(N_SRC_WIN)]
                        banks = [
                            psum_pool.tile([128, D], fp32, tag="bank", name="bank")
                            for _ in range(W_PER_SUPER)
                        ]
                        for s in range(N_SRC_WIN):
                            col0 = (sup * N_SRC_WIN + s) * WT
                            indb = ind_pool.tile([128, WT, 128], fp16, tag="ind")
                            nc.vector.tensor_tensor(
                                indb[:, :, :],
                                iota_wt[:, :, :],
                                keys_sb[:, col0 : col0 + WT].broadcast_to(
                                    (128, WT, 128)
                                ),
                                mybir.AluOpType.is_equal,
                            )
                            for wi in range(W_PER_SUPER):
                                w = sup * W_PER_SUPER + wi
                                bank = banks[wi]
                                for t in range(T_PER_BUCKET):
                                    nc.tensor.matmul(
                                        bank[:, :],
                                        indb[:, wi * T_PER_BUCKET + t, :],
                                        chunks[s][:, wi * T_PER_BUCKET + t, 0:D],
                                        start=(s == 0 and t == 0),
                                        stop=False,
                                    )
                                if s == N_SRC_WIN - 1:
                                    nc.tensor.matmul(
                                        bank[:, :],
                                        ident_sb[:, :],
                                        staged[:, w, 0:D],
                                        start=False,
                                        stop=True,
                                    )
                        for wi in range(W_PER_SUPER):
                            w = sup * W_PER_SUPER + wi
                            nc.scalar.activation(
                                g32[:, w, :],
                                banks[wi][:, :],
                                mybir.ActivationFunctionType.Copy,
                                scale=n2_sb[:, w : w + 1],
                            )
                            if w % CHUNK_W == CHUNK_W - 1:
                                q = w // CHUNK_W
                                c0 = q * CHUNK_W
                                if h < K_HOPS:
                                    nc.scalar.activation(
                                        staged[:, c0 : c0 + CHUNK_W, 0:D],
                                        g32[:, c0 : c0 + CHUNK_W, :],
                                        mybir.ActivationFunctionType.Copy,
                                    )
                                    nc.sync.dma_start(
                                        out=cc_in[h][
                                            c0 * 128 : (c0 + CHUNK_W) * 128, :
                                        ].rearrange("(w p) f -> p w f", p=128),
                                        in_=staged[:, c0 : c0 + CHUNK_W, :],
                                    )
                                    cc_allgather(
                                        cc_in[h][c0 * 128 : (c0 + CHUNK_W) * 128, :],
                                        cc_out[h][q * SRC_WIN : (q + 1) * SRC_WIN, :],
                                    )
                                gating_chunk(q)

            # ---- output: out = sqd * acc (reuse g32 as fp32 staging) ----
            nc.vector.tensor_tensor(
                g32[:, :, :],
                acc[:, :, :],
                sqd_sb[:, :]
                .rearrange("p (w one) -> p w one", one=1)
                .broadcast_to((128, W_PER_CORE, D)),
                mybir.AluOpType.mult,
            )
            nc.sync.dma_start(
                out=out_pm[:, :].rearrange("(w p) f -> p w f", p=128),
                in_=g32[:, :, :],
            )

    nc.finalize()
    return nc


# ----------------------------------------------------------------------------
# Entry point
# ----------------------------------------------------------------------------
_CACHED = {}


def kernel(**inputs):
    feats = np.asarray(inputs["feats"], dtype=np.float32)
    s = np.asarray(inputs["s"], dtype=np.float32)
    src = np.asarray(inputs["src"])
    dst = np.asarray(inputs["dst"])

    in_maps, meta = _preprocess(feats, s, src, dst)

    from concourse.bass_utils import run_bass_kernel_spmd

    nc = _CACHED.get("nc")
    if nc is None:
        nc = _build()
        _CACHED["nc"] = nc

    res = run_bass_kernel_spmd(nc, in_maps, core_ids=list(range(N_CORES)))
    _CACHED["last_result"] = res
    # unshard: out_pm row for node at (core, window w, pos p) is w*128 + p
    out = np.zeros((N_NODES, D), dtype=np.float32)
    core_of, wloc_of, pos_of = meta["core_of"], meta["wloc_of"], meta["pos_of"]
    rows = wloc_of.astype(np.int64) * 128 + pos_of.astype(np.int64)
    for c in range(N_CORES):
        m = core_of == c
        out[m] = res.results[c]["out_pm"][rows[m]]
    return out


if __name__ == "__main__":
    nc = _build()
    print("build ok")
